# revision 44
# baseline (speedup 1.0000x reference)
"""DreamAttention sparse-attention kernel for 8 Trainium2 NeuronCores.

Sharding: tensor-parallel over heads. Core c owns kv-head c and q-heads
(2c, 2c+1). Each core projects q for all tokens (its head pair), projects
k/v for the salient rows (its kv head), applies RoPE, folds the
salient-row cache update into the attention math (see below), and runs
full bidirectional GQA attention for its heads. The per-head attention
outputs (kept in o^T layout) are re-sharded token-wise with an on-device
AllToAll, after which every core computes the full o_proj for its
512-token slice (so no all-reduce is needed); the host concatenates the
8 row slices.

Cache-update-without-scatter trick: the reference scatters freshly
projected k/v rows into the caches at idx_salient before attention. Here
the host instead zeroes the salient rows of the previous cache and of
the denominator-mask vector, so a stale key row contributes exp(q.0)=1
times a zero value to the numerator and is excluded from the softmax
denominator, while the new salient keys/values enter as an extra
1024-key block computed densely; new keys belonging to the other batch
are killed with a -60 additive bias inside exp(scale*x + bias). Every
device-side operation is a dense matmul/elementwise op with no
data-dependent indexing.

Matmul instructions are the cost floor (~280 ns each regardless of
moving rows), so the attention inner loop keeps every matmul at the
maximum 512-row moving size: per key tile it issues two score matmuls
(K-tile stationary), two PV matmuls (V-tile stationary, accumulating
o^T), and two denominator matmuls (host-built 0/1 mask stationary,
giving the softmax denominator as a 2-row PSUM accumulator). The
normalization 1/den is broadcast across partitions with a K=1 matmul
and applied to o^T before the AllToAll. Matmuls run in float32r (fp32
storage, full-rate PE; measured kernel rel err ~3e-4).
"""

import os
import sys

for _p in ("/opt/trn_rl_repo", "/root/.axon_site/_ro/trn_rl_repo"):
    if os.path.isdir(_p) and _p not in sys.path:
        sys.path.insert(0, _p)

import numpy as np
import ml_dtypes

import concourse.bacc as bacc
import concourse.mybir as mybir
import concourse.tile as tile
from concourse import bass_utils

B, L = 2, 2048
T = B * L
HIDDEN = 2048
H, HKV, D = 16, 8, 128
S = 1024
ROPE_BASE = 1000000.0
HALF = D // 2
N_CORES = 8
G = H // HKV              # q heads per core (= per kv head)
DOUT = G * D              # 256 q-proj cols per core
TPC = T // N_CORES        # 512 output token rows per core
NKT = HIDDEN // 128       # 16 contraction tiles
SCALE = float(D) ** -0.5
NEG = -60.0               # kills cross-batch salient keys inside exp

F32 = mybir.dt.float32
F32R = mybir.dt.float32r
BF16 = mybir.dt.bfloat16

_cache = {}


def _enable_ldw_opt():
    from concourse import compiler_utils
    flags = getattr(compiler_utils, "_COMPILER_FLAGS", None)
    try:
        cur = list(compiler_utils.get_compiler_flags())
    except Exception:
        return
    changed = False
    out = []
    for f in cur:
        if "--enable-ldw-opt=false" in f:
            out.append(f.replace("--enable-ldw-opt=false",
                                 "--enable-ldw-opt=true"))
            changed = True
        else:
            out.append(f)
    if changed:
        compiler_utils.set_compiler_flags(out)


def _rope_apply(nc, out_ap, x_ap, xsw_ap, cs1_ap, cs2_ap, tmp_ap):
    """NeoX rope in [d, token] layout, same-partition form.

    out = x * [cos;cos] + swap(x) * [-sin;sin], where swap(x) (the two
    d-halves exchanged) was produced by a PE matmul with a permutation
    matrix, so every DVE operand here starts at partition 0.
    """
    mul = mybir.AluOpType.mult
    add = mybir.AluOpType.add
    nc.vector.tensor_tensor(tmp_ap, xsw_ap, cs2_ap, mul)
    nc.vector.tensor_tensor(out_ap, x_ap, cs1_ap, mul)
    nc.vector.tensor_tensor(out_ap, out_ap, tmp_ap, add)


def _build():
    nc = bacc.Bacc("TRN2", target_bir_lowering=False, debug=False,
                   num_devices=N_CORES)

    NJT = S // 128            # 8 salient key tiles
    NST = L // 128            # 16 prev key tiles per batch
    NTOT = NST + NJT          # 24 key tiles per batch
    IC = 512                  # query chunk
    NIC = L // IC             # 4 chunks per batch
    NIT = TPC // 128          # 4 output row tiles

    # ---- DRAM I/O (per-core shards prepared by the host) ----
    hT = nc.dram_tensor("hT", [HIDDEN, T], BF16, kind="ExternalInput").ap()
    hsalT = nc.dram_tensor("hsalT", [HIDDEN + 1, S], F32R, kind="ExternalInput").ap()
    wq = nc.dram_tensor("wq", [128, NKT * DOUT], BF16, kind="ExternalInput").ap()
    bq = nc.dram_tensor("bq", [G, 128, 1], F32, kind="ExternalInput").ap()
    wk = nc.dram_tensor("wk", [128, NKT * D], F32R, kind="ExternalInput").ap()
    bk = nc.dram_tensor("bk", [128, 1], F32, kind="ExternalInput").ap()
    wv = nc.dram_tensor("wv", [128 + 1, NKT * D], F32R, kind="ExternalInput").ap()
    wo = nc.dram_tensor("wo", [HIDDEN, HIDDEN], F32R, kind="ExternalInput").ap()
    kpT = nc.dram_tensor("kpT", [B, D, L], BF16, kind="ExternalInput").ap()
    vpa = nc.dram_tensor("vpa", [B, L, D], F32R, kind="ExternalInput").ap()
    dmask = nc.dram_tensor("dmask", [B, 128, 2 * NTOT], F32R,
                           kind="ExternalInput").ap()
    onem = nc.dram_tensor("onem", [1, 128], F32R, kind="ExternalInput").ap()
    csq1 = nc.dram_tensor("csq1", [D, T], BF16, kind="ExternalInput").ap()
    csq2 = nc.dram_tensor("csq2", [D, T], BF16, kind="ExternalInput").ap()
    css1 = nc.dram_tensor("css1", [D, S], F32R, kind="ExternalInput").ap()
    css2 = nc.dram_tensor("css2", [D, S], F32R, kind="ExternalInput").ap()
    swm = nc.dram_tensor("swm", [D, D], BF16, kind="ExternalInput").ap()
    swmf = nc.dram_tensor("swmf", [D, D], F32R, kind="ExternalInput").ap()
    idm = nc.dram_tensor("idm", [D, D], F32R, kind="ExternalInput").ap()
    sbias = nc.dram_tensor("sbias", [B, 128, NJT], F32, kind="ExternalInput").ap()
    out = nc.dram_tensor("out", [TPC, HIDDEN], F32, kind="ExternalOutput").ap()

    Exp = mybir.ActivationFunctionType.Exp
    Copy = mybir.ActivationFunctionType.Copy

    with tile.TileContext(nc) as tc:
        with (
            tc.tile_pool(name="consts", bufs=1) as consts,
            tc.tile_pool(name="dram", bufs=1, space="DRAM") as dram,
        ):
            ident = consts.tile([128, 128], F32R)
            swm_t = consts.tile([D, D], BF16)
            swmf_t = consts.tile([D, D], F32R)
            onem_t = consts.tile([1, 128], F32R)
            css1_t = consts.tile([D, S], F32R)
            css2_t = consts.tile([D, S], F32R)
            sbias_t = [consts.tile([128, NJT], F32, name=f"sbias{b}")
                       for b in range(B)]
            dmask_t = [consts.tile([128, 2 * NTOT], F32R, name=f"dmask{b}")
                       for b in range(B)]
            bq_t = [consts.tile([128, 1], F32, name=f"bqt{g}") for g in range(G)]
            bk_t = consts.tile([128, 1], F32)

            # o^T stacked layout: block j (token chunk j) = rows
            # [j*DOUT, (j+1)*DOUT) = this core's two heads' o^T columns.
            a2a_in = [dram.tile([N_CORES * D, TPC // B], F32R,
                                name=f"a2a_in{i}") for i in range(G * B)]
            a2a_out = [dram.tile([N_CORES * D, TPC // B], F32R,
                                 name=f"a2a_out{i}") for i in range(G * B)]

            wost_cm = tc.tile_pool(name="wost", bufs=20)
            wost = wost_cm.__enter__()
            with (
                tc.tile_pool(name="wqp", bufs=1) as wqp,
                tc.tile_pool(name="wkvp", bufs=1) as wkvp,
                tc.tile_pool(name="kvres", bufs=1) as kvres,
                tc.tile_pool(name="qres", bufs=1) as qres,
            ):
                wk_s = wkvp.tile([128, NKT * D], F32R)
                wv_s = wkvp.tile([128, NKT * D], F32R)
                wv_last = wkvp.tile([1, D], F32R)
                half = NKT * D // 2
                nc.sync.dma_start(wk_s[:, 0:half], wk[:, 0:half])
                nc.scalar.dma_start(wk_s[:, half:], wk[:, half:])
                nc.sync.dma_start(wv_s[:, 0:half], wv[0:128, 0:half])
                nc.scalar.dma_start(wv_s[:, half:], wv[0:128, half:])
                nc.sync.dma_start(wv_last[:],
                                  wv[128:129, 0:D])
                wk_t = [wk_s[:, k * D:(k + 1) * D] for k in range(NKT)]
                wv_t = [wv_s[:, k * D:(k + 1) * D] for k in range(NKT)]
                wq_s = wqp.tile([128, NKT * DOUT], BF16)
                nc.gpsimd.dma_start(wq_s[:], wq[:])
                wq_t = [wq_s[:, k * DOUT:(k + 1) * DOUT] for k in range(NKT)]
                # small consts on the gpsimd queue so the sync queue can
                # start streaming hsalT immediately
                nc.gpsimd.dma_start(swm_t[:], swm[:])
                nc.gpsimd.dma_start(swmf_t[:], swmf[:])
                nc.gpsimd.dma_start(ident[:], idm[:])
                nc.gpsimd.dma_start(css1_t[:], css1[:])
                nc.gpsimd.dma_start(css2_t[:], css2[:])
                nc.gpsimd.dma_start(bk_t[:], bk[:])
                nc.gpsimd.dma_start(onem_t[:], onem[:])
                for g in range(G):
                    nc.gpsimd.dma_start(bq_t[g][:], bq[g])
                for b in range(B):
                    nc.gpsimd.dma_start(sbias_t[b][:], sbias[b])
                    nc.gpsimd.dma_start(dmask_t[b][:], dmask[b])


                # Residents: prev-cache K^T and V rows per batch, new K^T
                # (roped) and new V rows.
                kpT_t = [kvres.tile([D, L], BF16, name=f"kpTt{b}")
                         for b in range(B)]
                vpa_t = [kvres.tile([128, NST * D], F32R, name=f"vpat{b}")
                         for b in range(B)]
                for b in range(B):
                    nc.gpsimd.dma_start(kpT_t[b][:], kpT[b])
                    nc.gpsimd.dma_start(
                        vpa_t[b][:].rearrange("p (s d) -> p s d", d=D),
                        vpa[b].rearrange("(s p) d -> p s d", p=128))
                knT_t = kvres.tile([D, S], BF16)
                vnew_t = [kvres.tile([128, D], F32R, name=f"vnewt{j}")
                          for j in range(NJT)]

                # ---- prefetch the first S3 hidden chunk during S2 ----
                hstr_cm = tc.tile_pool(name="hstr", bufs=12)
                hstr = hstr_cm.__enter__()
                ht_pre = []
                for k in range(12):
                    ht = hstr.tile([128, 512], BF16, tag="ht", name=f"htp{k}")
                    eng = nc.sync if k % 2 == 0 else nc.scalar
                    eng.dma_start(ht[:], hT[k * 128:(k + 1) * 128, 0:512])
                    ht_pre.append(ht)

                # ---- S2: kv projection for salient rows ----
                with (
                    tc.tile_pool(name="hsal", bufs=6) as hsalp,
                    tc.tile_pool(name="s2sb", bufs=1) as s2sb,
                    tc.tile_pool(name="kvps", bufs=1, space="PSUM") as kvps,
                ):
                    kn_ps = kvps.tile([D, S], F32)
                    vt_ps = kvps.tile([D, S], F32)
                    for k in range(NKT):
                        hs = hsalp.tile([128, S], F32R, tag="hs")
                        heng = nc.sync if k % 2 == 0 else nc.scalar
                        heng.dma_start(hs[:], hsalT[k * 128:(k + 1) * 128, :])
                        for n in range(S // 512):
                            sl = slice(n * 512, (n + 1) * 512)
                            nc.tensor.matmul(kn_ps[:, sl], wk_t[k], hs[:, sl],
                                             start=(k == 0), stop=(k == NKT - 1))
                            nc.tensor.matmul(vt_ps[:, sl], wv_t[k], hs[:, sl],
                                             start=(k == 0), stop=False)
                    hlast = hsalp.tile([1, S], F32R, tag="hl")
                    nc.sync.dma_start(hlast[:], hsalT[HIDDEN:HIDDEN + 1, :])
                    for n in range(S // 512):
                        sl = slice(n * 512, (n + 1) * 512)
                        nc.tensor.matmul(vt_ps[:, sl], wv_last[:], hlast[:, sl],
                                         start=False, stop=True)
                    # K: bias then rope into knT_t
                    knraw = s2sb.tile([D, S], F32R)
                    nc.vector.tensor_scalar_add(knraw[:], kn_ps[:], bk_t[:, 0:1])
                    with tc.tile_pool(name="kswp", bufs=1, space="PSUM") as kswp:
                        ksw_ps = kswp.tile([D, S], F32)
                        for n in range(S // 512):
                            sl = slice(n * 512, (n + 1) * 512)
                            nc.tensor.matmul(ksw_ps[:, sl], swmf_t[:],
                                             knraw[:, sl], start=True, stop=True)
                        ktmp = s2sb.tile([D, S], F32R)
                        _rope_apply(nc, knT_t[:], knraw[:], ksw_ps[:],
                                    css1_t[:], css2_t[:], ktmp[:])
                    # V: copy V^T out of PSUM, transpose to row tiles
                    vtS = s2sb.tile([D, S], F32R)
                    nc.scalar.activation(vtS[:], vt_ps[:], Copy)
                    with tc.tile_pool(name="vtrp", bufs=2, space="PSUM") as vtrp:
                        for jt in range(NJT):
                            tp = vtrp.tile([128, 128], F32R, tag="tp")
                            nc.tensor.transpose(
                                tp[:], vtS[:, jt * 128:(jt + 1) * 128], ident[:])
                            nc.vector.tensor_copy(vnew_t[jt][:], tp[:])

                # ---- S3: q projection + rope ----
                qT_t = [qres.tile([D, T], BF16, name=f"qTt{g}") for g in range(G)]
                with (
                    tc.tile_pool(name="csqp", bufs=1) as csqp,
                    tc.tile_pool(name="qraw", bufs=4) as qrawp,
                    tc.tile_pool(name="qps", bufs=4, space="PSUM") as qps,
                    tc.tile_pool(name="qswps", bufs=2, space="PSUM") as qswps,
                ):
                    csq1_t = csqp.tile([D, T], BF16)
                    csq2_t = csqp.tile([D, T], BF16)
                    nc.gpsimd.dma_start(csq1_t[:], csq1[:])
                    nc.gpsimd.dma_start(csq2_t[:], csq2[:])
                    for n in range(T // 512):
                        sl = slice(n * 512, (n + 1) * 512)
                        q_ps = [qps.tile([128, 512], F32, tag="qp",
                                         name=f"qps{g}") for g in range(G)]
                        for k in range(NKT):
                            if n == 0 and k < 12:
                                ht = ht_pre[k]
                            else:
                                ht = hstr.tile([128, 512], BF16, tag="ht")
                                eng = nc.sync if k % 2 == 0 else nc.scalar
                                eng.dma_start(ht[:],
                                              hT[k * 128:(k + 1) * 128, sl])
                            for g in range(G):
                                nc.tensor.matmul(
                                    q_ps[g][:], wq_t[k][:, g * 128:(g + 1) * 128],
                                    ht[:], start=(k == 0), stop=(k == NKT - 1))
                        for g in range(G):
                            qraw = qrawp.tile([128, 512], BF16, tag="qr")
                            nc.vector.tensor_scalar_add(qraw[:], q_ps[g][:],
                                                        bq_t[g][:, 0:1])
                            qsw_ps = qswps.tile([128, 512], F32, tag="qsw")
                            nc.tensor.matmul(qsw_ps[:], swm_t[:], qraw[:],
                                             start=True, stop=True)
                            qtmp = qrawp.tile([128, 512], BF16, tag="qtmp")
                            _rope_apply(nc, qT_t[g][:, sl], qraw[:], qsw_ps[:],
                                        csq1_t[:, sl], csq2_t[:, sl], qtmp[:])

                hstr_cm.__exit__(None, None, None)

                # ---- S4: attention, o^T accumulated V-stationary ----
                # prefetch the first o_proj weight block during attention
                wo_t = {}
                for dt in range(NKT):
                    w = wost.tile([128, 512], F32R, tag="wot")
                    nc.sync.dma_start(
                        w[:], wo[dt * 128:(dt + 1) * 128, 0:512])
                    wo_t[(0, dt)] = w
                with (
                    tc.tile_pool(name="ptp", bufs=6) as ptp,
                    tc.tile_pool(name="oscp", bufs=8) as oscp,
                    tc.tile_pool(name="rcp", bufs=8) as rcpp,
                    tc.tile_pool(name="scps", bufs=3, space="PSUM") as scps,
                    tc.tile_pool(name="opps", bufs=2, space="PSUM") as opps,
                    tc.tile_pool(name="dnps", bufs=2, space="PSUM") as dnps,
                ):
                    for g in range(G):
                        for b in range(B):
                            for icp in range(NIC // 2):
                                ics = (2 * icp, 2 * icp + 1)
                                qsls = [slice(b * L + ic * IC,
                                              b * L + (ic + 1) * IC)
                                        for ic in ics]
                                op_ps = [opps.tile([128, IC], F32, tag="op",
                                                   name=f"op{x}")
                                         for x in range(2)]
                                dn_ps = [dnps.tile([2, IC], F32, tag="dn",
                                                   name=f"dn{x}")
                                         for x in range(2)]
                                for st in range(NTOT):
                                    if st < NST:
                                        ktile = kpT_t[b][:, st * 128:(st + 1) * 128]
                                        vtile = vpa_t[b][:, st * D:(st + 1) * D]
                                    else:
                                        jt = st - NST
                                        ktile = knT_t[:, jt * 128:(jt + 1) * 128]
                                        vtile = vnew_t[jt][:]
                                    pts = []
                                    for x in range(2):
                                        sc = scps.tile([128, IC], F32, tag="sc")
                                        nc.tensor.matmul(sc[:], ktile,
                                                         qT_t[g][:, qsls[x]],
                                                         start=True, stop=True)
                                        pt = ptp.tile([128, IC], F32R, tag="pt")
                                        if st < NST:
                                            nc.scalar.activation(pt[:], sc[:],
                                                                 Exp, scale=SCALE)
                                        else:
                                            nc.scalar.activation(
                                                pt[:], sc[:], Exp, scale=SCALE,
                                                bias=sbias_t[b][:, jt:jt + 1])
                                        pts.append(pt)
                                    for x in range(2):
                                        nc.tensor.matmul(op_ps[x][:], vtile,
                                                         pts[x][:],
                                                         start=(st == 0),
                                                         stop=(st == NTOT - 1))
                                    dmt = dmask_t[b][:, st * 2:(st + 1) * 2]
                                    for x in range(2):
                                        nc.tensor.matmul(dn_ps[x][:], dmt,
                                                         pts[x][:],
                                                         start=(st == 0),
                                                         stop=(st == NTOT - 1))
                                for x in range(2):
                                    op_s = oscp.tile([128, IC], F32R, tag="opc")
                                    nc.vector.tensor_copy(op_s[:], op_ps[x][:])
                                    rc = rcpp.tile([1, IC], F32R, tag="rc")
                                    with nc.allow_low_precision(
                                            reason="float32r stores fp32 bits"):
                                        nc.vector.reciprocal(rc[:],
                                                             dn_ps[x][0:1, :])
                                    rb_s = oscp.tile([128, IC], F32R, tag="rbs")
                                    nc.gpsimd.partition_broadcast(
                                        rb_s[:], rc[0:1, :])
                                    osc = oscp.tile([128, IC], F32R, tag="osc")
                                    nc.vector.tensor_tensor(
                                        osc[:], op_s[:], rb_s[:],
                                        mybir.AluOpType.mult)
                                    buf = a2a_in[g * B + b]
                                    hwc = TPC // B
                                    for hh in range(2):
                                        r0 = (2 * ics[x] + hh) * D
                                        nc.sync.dma_start(
                                            buf[r0:r0 + D, :],
                                            osc[:, hh * hwc:(hh + 1) * hwc])
                            # token re-shard for (g, b); runs on the
                            # TOPSP/SDMA path while the PE keeps computing.
                            nc.gpsimd.collective_compute(
                                "AllToAll", mybir.AluOpType.bypass,
                                ins=[a2a_in[g * B + b].opt()],
                                outs=[a2a_out[g * B + b].opt()],
                                replica_groups=[list(range(N_CORES))],
                            )

            # ---- S6: o_proj for this core's 512 token rows ----
            with (
                tc.tile_pool(name="oTp", bufs=1) as oTp,
                tc.tile_pool(name="outsb", bufs=4) as outsbp,
                tc.tile_pool(name="opps2", bufs=2, space="PSUM") as opps2,
            ):
                oT_s = [oTp.tile([128, TPC], F32R, name=f"oTs{dt}")
                        for dt in range(NKT)]
                hwc = TPC // B
                for dt in range(NKT):
                    j, g = dt // G, dt % G
                    for b in range(B):
                        nc.sync.dma_start(
                            oT_s[dt][:, b * hwc:(b + 1) * hwc],
                            a2a_out[g * B + b][j * 128:(j + 1) * 128, :])
                for hc in range(1, HIDDEN // 512):
                    for dt in range(NKT):
                        w = wost.tile([128, 512], F32R, tag="wot")
                        nc.sync.dma_start(
                            w[:], wo[dt * 128:(dt + 1) * 128,
                                     hc * 512:(hc + 1) * 512])
                        wo_t[(hc, dt)] = w
                for hc in range(HIDDEN // 512):
                    for it in range(NIT):
                        op_ps = opps2.tile([128, 512], F32, tag="oo")
                        for dt in range(NKT):
                            nc.tensor.matmul(
                                op_ps[:],
                                oT_s[dt][:, it * 128:(it + 1) * 128],
                                wo_t[(hc, dt)][:],
                                start=(dt == 0), stop=(dt == NKT - 1))
                        ob = outsbp.tile([128, 512], F32, tag="ob")
                        nc.scalar.activation(ob[:], op_ps[:], Copy)
                        nc.sync.dma_start(
                            out[it * 128:(it + 1) * 128,
                                hc * 512:(hc + 1) * 512], ob[:])
            wost_cm.__exit__(None, None, None)

    nc.compile()
    return nc


def kernel(positions, hidden_states, idx_salient, k_cache_prev, v_cache_prev,
           Wq, bq, Wkv, bkv, Wo):
    pos = np.asarray(positions).astype(np.int64)
    hs = np.asarray(hidden_states, dtype=np.float32)
    idx = np.asarray(idx_salient).astype(np.int64)
    kc = np.asarray(k_cache_prev, dtype=np.float32)
    vc = np.asarray(v_cache_prev, dtype=np.float32)
    Wq = np.asarray(Wq, dtype=np.float32)
    bq = np.asarray(bq, dtype=np.float32)
    Wkv = np.asarray(Wkv, dtype=np.float32)
    bkv = np.asarray(bkv, dtype=np.float32)
    Wo = np.asarray(Wo, dtype=np.float32)

    if "nc" not in _cache:
        _enable_ldw_opt()
        _cache["nc"] = _build()
    nc = _cache["nc"]

    NST = L // 128
    NJT = S // 128
    NTOT = NST + NJT

    hT = np.ascontiguousarray(hs.T).astype(ml_dtypes.bfloat16)
    hsalT = np.concatenate([np.ascontiguousarray(hs[idx].T),
                            np.ones((1, S), np.float32)], axis=0)
    inv_freq = 1.0 / (ROPE_BASE ** (np.arange(HALF, dtype=np.float64) / HALF))
    ang_q = np.outer(inv_freq, pos.astype(np.float64))
    csq1_h = np.concatenate([np.cos(ang_q), np.cos(ang_q)]).astype(ml_dtypes.bfloat16)
    csq2_h = np.concatenate([-np.sin(ang_q), np.sin(ang_q)]).astype(ml_dtypes.bfloat16)
    ang_s = np.outer(inv_freq, pos[idx].astype(np.float64))
    css1_h = np.concatenate([np.cos(ang_s), np.cos(ang_s)]).astype(np.float32)
    css2_h = np.concatenate([-np.sin(ang_s), np.sin(ang_s)]).astype(np.float32)
    swm_h = np.zeros((D, D), np.float32)
    swm_h[np.arange(D), (np.arange(D) + HALF) % D] = 1.0
    batch_of_j = (idx // L).astype(np.int64)
    kv_size = HKV * D

    # denominator mask: per batch, per key tile, 0 for zeroed (stale
    # salient) prev-cache rows, 1 otherwise; all-ones for new-key tiles.
    keep = np.ones(T, np.float32)
    keep[idx] = 0.0
    dmask_h = np.empty((B, 128, 2 * NTOT), np.float32)
    for b in range(B):
        kb = keep[b * L:(b + 1) * L].reshape(NST, 128).T   # [128, 16]
        dmask_h[b, :, :2 * NST] = np.repeat(kb, 2, axis=1)
        dmask_h[b, :, 2 * NST:] = 1.0

    sb_h = np.stack([
        np.where(batch_of_j == b, 0.0, NEG).astype(np.float32)
          .reshape(NJT, 128).T
        for b in range(B)])

    in_maps = []
    for c in range(N_CORES):
        kcc = kc[:, c, :].copy()
        kcc[idx] = 0.0
        kpT_h = np.stack([np.ascontiguousarray(kcc[b * L:(b + 1) * L].T)
                          for b in range(B)]).astype(ml_dtypes.bfloat16)
        vcc = vc[:, c, :].copy()
        vcc[idx] = 0.0
        vpa_h = np.stack([vcc[b * L:(b + 1) * L] for b in range(B)])
        in_maps.append({
            "hT": hT,
            "hsalT": hsalT,
            "wq": np.ascontiguousarray(
                Wq[:, c * DOUT:(c + 1) * DOUT].reshape(NKT, 128, DOUT)
                .transpose(1, 0, 2).reshape(128, NKT * DOUT))
                .astype(ml_dtypes.bfloat16),
            "bq": np.ascontiguousarray(
                bq[c * DOUT:(c + 1) * DOUT].reshape(G, 128, 1)),
            "wk": np.ascontiguousarray(
                Wkv[:, c * D:(c + 1) * D].reshape(NKT, 128, D)
                .transpose(1, 0, 2).reshape(128, NKT * D)),
            "bk": np.ascontiguousarray(bkv[c * D:(c + 1) * D].reshape(128, 1)),
            "wv": np.concatenate([
                Wkv[:, kv_size + c * D:kv_size + (c + 1) * D]
                .reshape(NKT, 128, D).transpose(1, 0, 2).reshape(128, NKT * D),
                np.pad(bkv[kv_size + c * D:kv_size + (c + 1) * D]
                       .reshape(1, D), ((0, 0), (0, (NKT - 1) * D)))],
                axis=0),
            "wo": Wo,
            "kpT": kpT_h,
            "vpa": vpa_h,
            "dmask": dmask_h,
            "onem": np.ones((1, 128), np.float32),
            "csq1": csq1_h,
            "csq2": csq2_h,
            "css1": css1_h,
            "css2": css2_h,
            "swm": swm_h.astype(ml_dtypes.bfloat16),
            "swmf": swm_h,
            "idm": np.eye(D, dtype=np.float32),
            "sbias": sb_h,
        })

    res = bass_utils.run_bass_kernel_spmd(nc, in_maps,
                                          core_ids=list(range(N_CORES)))
    # core c's "out" rows: [0:256] = batch-0 tokens c*256.., [256:512] =
    # batch-1 tokens 2048 + c*256..
    half = TPC // B
    full = np.empty((T, HIDDEN), np.float32)
    for c in range(N_CORES):
        o = res.results[c]["out"]
        full[c * half:(c + 1) * half] = o[0:half]
        full[L + c * half:L + (c + 1) * half] = o[half:TPC]
    return full


# revision 45
# speedup vs baseline: 1.0257x; 1.0257x over previous
"""DreamAttention sparse-attention kernel for 8 Trainium2 NeuronCores.

Sharding: tensor-parallel over heads. Core c owns kv-head c and q-heads
(2c, 2c+1). Each core projects q for all tokens (its head pair), projects
k/v for the salient rows (its kv head), applies RoPE, folds the
salient-row cache update into the attention math (see below), and runs
full bidirectional GQA attention for its heads. The per-head attention
outputs (kept in o^T layout) are re-sharded token-wise with an on-device
AllToAll, after which every core computes the full o_proj for its
512-token slice (so no all-reduce is needed); the host concatenates the
8 row slices.

Cache-update-without-scatter trick: the reference scatters freshly
projected k/v rows into the caches at idx_salient before attention. Here
the host instead zeroes the salient rows of the previous cache and of
the denominator-mask vector, so a stale key row contributes exp(q.0)=1
times a zero value to the numerator and is excluded from the softmax
denominator, while the new salient keys/values enter as an extra
1024-key block computed densely; new keys belonging to the other batch
are killed with a -60 additive bias inside exp(scale*x + bias). Every
device-side operation is a dense matmul/elementwise op with no
data-dependent indexing.

Matmul instructions are the cost floor (~280 ns each regardless of
moving rows), so the attention inner loop keeps every matmul at the
maximum 512-row moving size: per key tile it issues two score matmuls
(K-tile stationary), two PV matmuls (V-tile stationary, accumulating
o^T), and two denominator matmuls (host-built 0/1 mask stationary,
giving the softmax denominator as a 2-row PSUM accumulator). The
normalization 1/den is broadcast across partitions with a K=1 matmul
and applied to o^T before the AllToAll. Matmuls run in float32r (fp32
storage, full-rate PE; measured kernel rel err ~3e-4).
"""

import os
import sys

for _p in ("/opt/trn_rl_repo", "/root/.axon_site/_ro/trn_rl_repo"):
    if os.path.isdir(_p) and _p not in sys.path:
        sys.path.insert(0, _p)

import numpy as np
import ml_dtypes

import concourse.bacc as bacc
import concourse.mybir as mybir
import concourse.tile as tile
from concourse import bass_utils

B, L = 2, 2048
T = B * L
HIDDEN = 2048
H, HKV, D = 16, 8, 128
S = 1024
ROPE_BASE = 1000000.0
HALF = D // 2
N_CORES = 8
G = H // HKV              # q heads per core (= per kv head)
DOUT = G * D              # 256 q-proj cols per core
TPC = T // N_CORES        # 512 output token rows per core
NKT = HIDDEN // 128       # 16 contraction tiles
SCALE = float(D) ** -0.5
NEG = -60.0               # kills cross-batch salient keys inside exp

F32 = mybir.dt.float32
F32R = mybir.dt.float32r
BF16 = mybir.dt.bfloat16

_cache = {}


def _enable_ldw_opt():
    from concourse import compiler_utils
    flags = getattr(compiler_utils, "_COMPILER_FLAGS", None)
    try:
        cur = list(compiler_utils.get_compiler_flags())
    except Exception:
        return
    changed = False
    out = []
    for f in cur:
        if "--enable-ldw-opt=false" in f:
            out.append(f.replace("--enable-ldw-opt=false",
                                 "--enable-ldw-opt=true"))
            changed = True
        else:
            out.append(f)
    if changed:
        compiler_utils.set_compiler_flags(out)


def _rope_apply(nc, out_ap, x_ap, xsw_ap, cs1_ap, cs2_ap, tmp_ap):
    """NeoX rope in [d, token] layout, same-partition form.

    out = x * [cos;cos] + swap(x) * [-sin;sin], where swap(x) (the two
    d-halves exchanged) was produced by a PE matmul with a permutation
    matrix, so every DVE operand here starts at partition 0.
    """
    mul = mybir.AluOpType.mult
    add = mybir.AluOpType.add
    nc.vector.tensor_tensor(tmp_ap, xsw_ap, cs2_ap, mul)
    nc.vector.tensor_tensor(out_ap, x_ap, cs1_ap, mul)
    nc.vector.tensor_tensor(out_ap, out_ap, tmp_ap, add)


def _build():
    nc = bacc.Bacc("TRN2", target_bir_lowering=False, debug=False,
                   num_devices=N_CORES)

    NJT = S // 128            # 8 salient key tiles
    NST = L // 128            # 16 prev key tiles per batch
    NTOT = NST + NJT          # 24 key tiles per batch
    IC = 512                  # query chunk
    NIC = L // IC             # 4 chunks per batch
    NIT = TPC // 128          # 4 output row tiles

    # ---- DRAM I/O (per-core shards prepared by the host) ----
    hT = nc.dram_tensor("hT", [HIDDEN, T], BF16, kind="ExternalInput").ap()
    hsalT = nc.dram_tensor("hsalT", [HIDDEN + 1, S], F32R, kind="ExternalInput").ap()
    wq = nc.dram_tensor("wq", [128, NKT * DOUT], BF16, kind="ExternalInput").ap()
    bq = nc.dram_tensor("bq", [G, 128, 1], F32, kind="ExternalInput").ap()
    wk = nc.dram_tensor("wk", [128, NKT * D], F32R, kind="ExternalInput").ap()
    bk = nc.dram_tensor("bk", [128, 1], F32, kind="ExternalInput").ap()
    wv = nc.dram_tensor("wv", [128 + 1, NKT * D], F32R, kind="ExternalInput").ap()
    wo = nc.dram_tensor("wo", [HIDDEN, HIDDEN], F32R, kind="ExternalInput").ap()
    kpT = nc.dram_tensor("kpT", [B, D, L], BF16, kind="ExternalInput").ap()
    vpa = nc.dram_tensor("vpa", [B, L, D], F32R, kind="ExternalInput").ap()
    dmask = nc.dram_tensor("dmask", [B, 128, 2 * NTOT], F32R,
                           kind="ExternalInput").ap()
    onem = nc.dram_tensor("onem", [1, 128], F32R, kind="ExternalInput").ap()
    csq1 = nc.dram_tensor("csq1", [D, T], BF16, kind="ExternalInput").ap()
    csq2 = nc.dram_tensor("csq2", [D, T], BF16, kind="ExternalInput").ap()
    css1 = nc.dram_tensor("css1", [D, S], F32R, kind="ExternalInput").ap()
    css2 = nc.dram_tensor("css2", [D, S], F32R, kind="ExternalInput").ap()
    swm = nc.dram_tensor("swm", [D, D], BF16, kind="ExternalInput").ap()
    swmf = nc.dram_tensor("swmf", [D, D], F32R, kind="ExternalInput").ap()
    idm = nc.dram_tensor("idm", [D, D], F32R, kind="ExternalInput").ap()
    sbias = nc.dram_tensor("sbias", [B, 128, NJT], F32, kind="ExternalInput").ap()
    out = nc.dram_tensor("out", [TPC, HIDDEN], F32, kind="ExternalOutput").ap()

    Exp = mybir.ActivationFunctionType.Exp
    Copy = mybir.ActivationFunctionType.Copy

    with tile.TileContext(nc) as tc:
        with (
            tc.tile_pool(name="consts", bufs=1) as consts,
            tc.tile_pool(name="dram", bufs=1, space="DRAM") as dram,
        ):
            ident = consts.tile([128, 128], F32R)
            swm_t = consts.tile([D, D], BF16)
            swmf_t = consts.tile([D, D], F32R)
            onem_t = consts.tile([1, 128], F32R)
            css1_t = consts.tile([D, S], F32R)
            css2_t = consts.tile([D, S], F32R)
            sbias_t = [consts.tile([128, NJT], F32, name=f"sbias{b}")
                       for b in range(B)]
            dmask_t = [consts.tile([128, 2 * NTOT], F32R, name=f"dmask{b}")
                       for b in range(B)]
            bq_t = [consts.tile([128, 1], F32, name=f"bqt{g}") for g in range(G)]
            bk_t = consts.tile([128, 1], F32)

            # o^T stacked layout: block j (token chunk j) = rows
            # [j*DOUT, (j+1)*DOUT) = this core's two heads' o^T columns.
            a2a_in = [dram.tile([N_CORES * D, TPC // B], F32R,
                                name=f"a2a_in{i}") for i in range(G * B)]
            a2a_out = [dram.tile([N_CORES * D, TPC // B], F32R,
                                 name=f"a2a_out{i}") for i in range(G * B)]

            wost_cm = tc.tile_pool(name="wost", bufs=20)
            wost = wost_cm.__enter__()
            with (
                tc.tile_pool(name="wqp", bufs=1) as wqp,
                tc.tile_pool(name="wkvp", bufs=1) as wkvp,
                tc.tile_pool(name="kvres", bufs=1) as kvres,
                tc.tile_pool(name="qres", bufs=1) as qres,
            ):
                wk_s = wkvp.tile([128, NKT * D], F32R)
                wv_s = wkvp.tile([128, NKT * D], F32R)
                wv_last = wkvp.tile([1, D], F32R)
                half = NKT * D // 2
                nc.sync.dma_start(wk_s[:, 0:half], wk[:, 0:half])
                nc.scalar.dma_start(wk_s[:, half:], wk[:, half:])
                nc.sync.dma_start(wv_s[:, 0:half], wv[0:128, 0:half])
                nc.scalar.dma_start(wv_s[:, half:], wv[0:128, half:])
                nc.sync.dma_start(wv_last[:],
                                  wv[128:129, 0:D])
                wk_t = [wk_s[:, k * D:(k + 1) * D] for k in range(NKT)]
                wv_t = [wv_s[:, k * D:(k + 1) * D] for k in range(NKT)]
                wq_s = wqp.tile([128, NKT * DOUT], BF16)
                nc.gpsimd.dma_start(wq_s[:], wq[:])
                wq_t = [wq_s[:, k * DOUT:(k + 1) * DOUT] for k in range(NKT)]
                # small consts on the gpsimd queue so the sync queue can
                # start streaming hsalT immediately
                nc.gpsimd.dma_start(swm_t[:], swm[:])
                nc.gpsimd.dma_start(swmf_t[:], swmf[:])
                nc.gpsimd.dma_start(ident[:], idm[:])
                nc.gpsimd.dma_start(css1_t[:], css1[:])
                nc.gpsimd.dma_start(css2_t[:], css2[:])
                nc.gpsimd.dma_start(bk_t[:], bk[:])
                nc.gpsimd.dma_start(onem_t[:], onem[:])
                for g in range(G):
                    nc.gpsimd.dma_start(bq_t[g][:], bq[g])
                for b in range(B):
                    nc.gpsimd.dma_start(sbias_t[b][:], sbias[b])
                    nc.gpsimd.dma_start(dmask_t[b][:], dmask[b])


                # Residents: prev-cache K^T and V rows per batch, new K^T
                # (roped) and new V rows.
                kpT_t = [kvres.tile([D, L], BF16, name=f"kpTt{b}")
                         for b in range(B)]
                vpa_t = [kvres.tile([128, NST * D], F32R, name=f"vpat{b}")
                         for b in range(B)]
                for b in range(B):
                    nc.gpsimd.dma_start(kpT_t[b][:], kpT[b])
                    nc.gpsimd.dma_start(
                        vpa_t[b][:].rearrange("p (s d) -> p s d", d=D),
                        vpa[b].rearrange("(s p) d -> p s d", p=128))
                knT_t = kvres.tile([D, S], BF16)
                vnew_t = [kvres.tile([128, D], F32R, name=f"vnewt{j}")
                          for j in range(NJT)]

                # ---- prefetch the first S3 hidden chunk during S2 ----
                hstr_cm = tc.tile_pool(name="hstr", bufs=12)
                hstr = hstr_cm.__enter__()
                ht_pre = []
                for k in range(12):
                    ht = hstr.tile([128, 512], BF16, tag="ht", name=f"htp{k}")
                    eng = nc.sync if k % 2 == 0 else nc.scalar
                    eng.dma_start(ht[:], hT[k * 128:(k + 1) * 128, 0:512])
                    ht_pre.append(ht)

                # ---- S2: kv projection for salient rows ----
                with (
                    tc.tile_pool(name="hsal", bufs=6) as hsalp,
                    tc.tile_pool(name="s2sb", bufs=1) as s2sb,
                    tc.tile_pool(name="kvps", bufs=1, space="PSUM") as kvps,
                ):
                    kn_ps = kvps.tile([D, S], F32)
                    vt_ps = kvps.tile([D, S], F32)
                    for k in range(NKT):
                        hs = hsalp.tile([128, S], F32R, tag="hs")
                        heng = nc.sync if k % 2 == 0 else nc.scalar
                        heng.dma_start(hs[:], hsalT[k * 128:(k + 1) * 128, :])
                        for n in range(S // 512):
                            sl = slice(n * 512, (n + 1) * 512)
                            nc.tensor.matmul(kn_ps[:, sl], wk_t[k], hs[:, sl],
                                             start=(k == 0), stop=(k == NKT - 1))
                            nc.tensor.matmul(vt_ps[:, sl], wv_t[k], hs[:, sl],
                                             start=(k == 0), stop=False)
                    hlast = hsalp.tile([1, S], F32R, tag="hl")
                    nc.sync.dma_start(hlast[:], hsalT[HIDDEN:HIDDEN + 1, :])
                    for n in range(S // 512):
                        sl = slice(n * 512, (n + 1) * 512)
                        nc.tensor.matmul(vt_ps[:, sl], wv_last[:], hlast[:, sl],
                                         start=False, stop=True)
                    # K: bias then rope into knT_t
                    knraw = s2sb.tile([D, S], F32R)
                    nc.vector.tensor_scalar_add(knraw[:], kn_ps[:], bk_t[:, 0:1])
                    with tc.tile_pool(name="kswp", bufs=1, space="PSUM") as kswp:
                        ksw_ps = kswp.tile([D, S], F32)
                        for n in range(S // 512):
                            sl = slice(n * 512, (n + 1) * 512)
                            nc.tensor.matmul(ksw_ps[:, sl], swmf_t[:],
                                             knraw[:, sl], start=True, stop=True)
                        ktmp = s2sb.tile([D, S], F32R)
                        _rope_apply(nc, knT_t[:], knraw[:], ksw_ps[:],
                                    css1_t[:], css2_t[:], ktmp[:])
                    # V: copy V^T out of PSUM, transpose to row tiles
                    vtS = s2sb.tile([D, S], F32R)
                    nc.scalar.activation(vtS[:], vt_ps[:], Copy)
                    with tc.tile_pool(name="vtrp", bufs=2, space="PSUM") as vtrp:
                        for jt in range(NJT):
                            tp = vtrp.tile([128, 128], F32R, tag="tp")
                            nc.tensor.transpose(
                                tp[:], vtS[:, jt * 128:(jt + 1) * 128], ident[:])
                            nc.vector.tensor_copy(vnew_t[jt][:], tp[:])

                # ---- S3: q projection + rope ----
                qT_t = [qres.tile([D, T], BF16, name=f"qTt{g}") for g in range(G)]
                with (
                    tc.tile_pool(name="csqp", bufs=1) as csqp,
                    tc.tile_pool(name="qraw", bufs=4) as qrawp,
                    tc.tile_pool(name="qps", bufs=4, space="PSUM") as qps,
                    tc.tile_pool(name="qswps", bufs=2, space="PSUM") as qswps,
                ):
                    csq1_t = csqp.tile([D, T], BF16)
                    csq2_t = csqp.tile([D, T], BF16)
                    nc.gpsimd.dma_start(csq1_t[:], csq1[:])
                    nc.gpsimd.dma_start(csq2_t[:], csq2[:])
                    for n in range(T // 512):
                        sl = slice(n * 512, (n + 1) * 512)
                        q_ps = [qps.tile([128, 512], F32, tag="qp",
                                         name=f"qps{g}") for g in range(G)]
                        for k in range(NKT):
                            if n == 0 and k < 12:
                                ht = ht_pre[k]
                            else:
                                ht = hstr.tile([128, 512], BF16, tag="ht")
                                eng = nc.sync if k % 2 == 0 else nc.scalar
                                eng.dma_start(ht[:],
                                              hT[k * 128:(k + 1) * 128, sl])
                            for g in range(G):
                                nc.tensor.matmul(
                                    q_ps[g][:], wq_t[k][:, g * 128:(g + 1) * 128],
                                    ht[:], start=(k == 0), stop=(k == NKT - 1))
                        for g in range(G):
                            qraw = qrawp.tile([128, 512], BF16, tag="qr")
                            nc.vector.tensor_scalar_add(qraw[:], q_ps[g][:],
                                                        bq_t[g][:, 0:1])
                            qsw_ps = qswps.tile([128, 512], F32, tag="qsw")
                            nc.tensor.matmul(qsw_ps[:], swm_t[:], qraw[:],
                                             start=True, stop=True)
                            qtmp = qrawp.tile([128, 512], BF16, tag="qtmp")
                            _rope_apply(nc, qT_t[g][:, sl], qraw[:], qsw_ps[:],
                                        csq1_t[:, sl], csq2_t[:, sl], qtmp[:])

                hstr_cm.__exit__(None, None, None)

                # ---- S4: attention, o^T accumulated V-stationary ----
                # prefetch the first o_proj weight block during attention
                wo_t = {}
                for dt in range(NKT):
                    w = wost.tile([128, 512], F32R, tag="wot")
                    nc.sync.dma_start(
                        w[:], wo[dt * 128:(dt + 1) * 128, 0:512])
                    wo_t[(0, dt)] = w
                with (
                    tc.tile_pool(name="ptp", bufs=6) as ptp,
                    tc.tile_pool(name="oscp", bufs=8) as oscp,
                    tc.tile_pool(name="rcp", bufs=8) as rcpp,
                    tc.tile_pool(name="scps", bufs=4, space="PSUM") as scps,
                    tc.tile_pool(name="opps", bufs=2, space="PSUM") as opps,
                    tc.tile_pool(name="dnps", bufs=2, space="PSUM") as dnps,
                ):
                    for g in range(G):
                        for b in range(B):
                            for icp in range(NIC // 2):
                                ics = (2 * icp, 2 * icp + 1)
                                qsls = [slice(b * L + ic * IC,
                                              b * L + (ic + 1) * IC)
                                        for ic in ics]
                                op_ps = [opps.tile([128, IC], F32, tag="op",
                                                   name=f"op{x}")
                                         for x in range(2)]
                                dn_ps = [dnps.tile([2, IC], F32, tag="dn",
                                                   name=f"dn{x}")
                                         for x in range(2)]
                                for st in range(NTOT):
                                    if st < NST:
                                        ktile = kpT_t[b][:, st * 128:(st + 1) * 128]
                                        vtile = vpa_t[b][:, st * D:(st + 1) * D]
                                    else:
                                        jt = st - NST
                                        ktile = knT_t[:, jt * 128:(jt + 1) * 128]
                                        vtile = vnew_t[jt][:]
                                    pts = []
                                    for x in range(2):
                                        sc = scps.tile([128, IC], F32, tag="sc")
                                        nc.tensor.matmul(sc[:], ktile,
                                                         qT_t[g][:, qsls[x]],
                                                         start=True, stop=True)
                                        pt = ptp.tile([128, IC], F32R, tag="pt")
                                        if st < NST:
                                            nc.scalar.activation(pt[:], sc[:],
                                                                 Exp, scale=SCALE)
                                        else:
                                            nc.scalar.activation(
                                                pt[:], sc[:], Exp, scale=SCALE,
                                                bias=sbias_t[b][:, jt:jt + 1])
                                        pts.append(pt)
                                    for x in range(2):
                                        nc.tensor.matmul(op_ps[x][:], vtile,
                                                         pts[x][:],
                                                         start=(st == 0),
                                                         stop=(st == NTOT - 1))
                                    dmt = dmask_t[b][:, st * 2:(st + 1) * 2]
                                    for x in range(2):
                                        nc.tensor.matmul(dn_ps[x][:], dmt,
                                                         pts[x][:],
                                                         start=(st == 0),
                                                         stop=(st == NTOT - 1))
                                for x in range(2):
                                    op_s = oscp.tile([128, IC], F32R, tag="opc")
                                    nc.vector.tensor_copy(op_s[:], op_ps[x][:])
                                    rc = rcpp.tile([1, IC], F32R, tag="rc")
                                    with nc.allow_low_precision(
                                            reason="float32r stores fp32 bits"):
                                        nc.vector.reciprocal(rc[:],
                                                             dn_ps[x][0:1, :])
                                    rb_s = oscp.tile([128, IC], F32R, tag="rbs")
                                    nc.gpsimd.partition_broadcast(
                                        rb_s[:], rc[0:1, :])
                                    osc = oscp.tile([128, IC], F32R, tag="osc")
                                    nc.vector.tensor_tensor(
                                        osc[:], op_s[:], rb_s[:],
                                        mybir.AluOpType.mult)
                                    buf = a2a_in[g * B + b]
                                    hwc = TPC // B
                                    for hh in range(2):
                                        r0 = (2 * ics[x] + hh) * D
                                        nc.sync.dma_start(
                                            buf[r0:r0 + D, :],
                                            osc[:, hh * hwc:(hh + 1) * hwc])
                            # token re-shard for (g, b); runs on the
                            # TOPSP/SDMA path while the PE keeps computing.
                            nc.gpsimd.collective_compute(
                                "AllToAll", mybir.AluOpType.bypass,
                                ins=[a2a_in[g * B + b].opt()],
                                outs=[a2a_out[g * B + b].opt()],
                                replica_groups=[list(range(N_CORES))],
                            )

            # ---- S6: o_proj for this core's 512 token rows ----
            with (
                tc.tile_pool(name="oTp", bufs=1) as oTp,
                tc.tile_pool(name="outsb", bufs=4) as outsbp,
                tc.tile_pool(name="opps2", bufs=2, space="PSUM") as opps2,
            ):
                oT_s = [oTp.tile([128, TPC], F32R, name=f"oTs{dt}")
                        for dt in range(NKT)]
                hwc = TPC // B
                for dt in range(NKT):
                    j, g = dt // G, dt % G
                    for b in range(B):
                        nc.sync.dma_start(
                            oT_s[dt][:, b * hwc:(b + 1) * hwc],
                            a2a_out[g * B + b][j * 128:(j + 1) * 128, :])
                for hc in range(1, HIDDEN // 512):
                    for dt in range(NKT):
                        w = wost.tile([128, 512], F32R, tag="wot")
                        nc.sync.dma_start(
                            w[:], wo[dt * 128:(dt + 1) * 128,
                                     hc * 512:(hc + 1) * 512])
                        wo_t[(hc, dt)] = w
                for hc in range(HIDDEN // 512):
                    for it in range(NIT):
                        op_ps = opps2.tile([128, 512], F32, tag="oo")
                        for dt in range(NKT):
                            nc.tensor.matmul(
                                op_ps[:],
                                oT_s[dt][:, it * 128:(it + 1) * 128],
                                wo_t[(hc, dt)][:],
                                start=(dt == 0), stop=(dt == NKT - 1))
                        ob = outsbp.tile([128, 512], F32, tag="ob")
                        nc.scalar.activation(ob[:], op_ps[:], Copy)
                        nc.sync.dma_start(
                            out[it * 128:(it + 1) * 128,
                                hc * 512:(hc + 1) * 512], ob[:])
            wost_cm.__exit__(None, None, None)

    nc.compile()
    return nc


def kernel(positions, hidden_states, idx_salient, k_cache_prev, v_cache_prev,
           Wq, bq, Wkv, bkv, Wo):
    pos = np.asarray(positions).astype(np.int64)
    hs = np.asarray(hidden_states, dtype=np.float32)
    idx = np.asarray(idx_salient).astype(np.int64)
    kc = np.asarray(k_cache_prev, dtype=np.float32)
    vc = np.asarray(v_cache_prev, dtype=np.float32)
    Wq = np.asarray(Wq, dtype=np.float32)
    bq = np.asarray(bq, dtype=np.float32)
    Wkv = np.asarray(Wkv, dtype=np.float32)
    bkv = np.asarray(bkv, dtype=np.float32)
    Wo = np.asarray(Wo, dtype=np.float32)

    if "nc" not in _cache:
        _enable_ldw_opt()
        _cache["nc"] = _build()
    nc = _cache["nc"]

    NST = L // 128
    NJT = S // 128
    NTOT = NST + NJT

    hT = np.ascontiguousarray(hs.T).astype(ml_dtypes.bfloat16)
    hsalT = np.concatenate([np.ascontiguousarray(hs[idx].T),
                            np.ones((1, S), np.float32)], axis=0)
    inv_freq = 1.0 / (ROPE_BASE ** (np.arange(HALF, dtype=np.float64) / HALF))
    ang_q = np.outer(inv_freq, pos.astype(np.float64))
    csq1_h = np.concatenate([np.cos(ang_q), np.cos(ang_q)]).astype(ml_dtypes.bfloat16)
    csq2_h = np.concatenate([-np.sin(ang_q), np.sin(ang_q)]).astype(ml_dtypes.bfloat16)
    ang_s = np.outer(inv_freq, pos[idx].astype(np.float64))
    css1_h = np.concatenate([np.cos(ang_s), np.cos(ang_s)]).astype(np.float32)
    css2_h = np.concatenate([-np.sin(ang_s), np.sin(ang_s)]).astype(np.float32)
    swm_h = np.zeros((D, D), np.float32)
    swm_h[np.arange(D), (np.arange(D) + HALF) % D] = 1.0
    batch_of_j = (idx // L).astype(np.int64)
    kv_size = HKV * D

    # denominator mask: per batch, per key tile, 0 for zeroed (stale
    # salient) prev-cache rows, 1 otherwise; all-ones for new-key tiles.
    keep = np.ones(T, np.float32)
    keep[idx] = 0.0
    dmask_h = np.empty((B, 128, 2 * NTOT), np.float32)
    for b in range(B):
        kb = keep[b * L:(b + 1) * L].reshape(NST, 128).T   # [128, 16]
        dmask_h[b, :, :2 * NST] = np.repeat(kb, 2, axis=1)
        dmask_h[b, :, 2 * NST:] = 1.0

    sb_h = np.stack([
        np.where(batch_of_j == b, 0.0, NEG).astype(np.float32)
          .reshape(NJT, 128).T
        for b in range(B)])

    in_maps = []
    for c in range(N_CORES):
        kcc = kc[:, c, :].copy()
        kcc[idx] = 0.0
        kpT_h = np.stack([np.ascontiguousarray(kcc[b * L:(b + 1) * L].T)
                          for b in range(B)]).astype(ml_dtypes.bfloat16)
        vcc = vc[:, c, :].copy()
        vcc[idx] = 0.0
        vpa_h = np.stack([vcc[b * L:(b + 1) * L] for b in range(B)])
        in_maps.append({
            "hT": hT,
            "hsalT": hsalT,
            "wq": np.ascontiguousarray(
                Wq[:, c * DOUT:(c + 1) * DOUT].reshape(NKT, 128, DOUT)
                .transpose(1, 0, 2).reshape(128, NKT * DOUT))
                .astype(ml_dtypes.bfloat16),
            "bq": np.ascontiguousarray(
                bq[c * DOUT:(c + 1) * DOUT].reshape(G, 128, 1)),
            "wk": np.ascontiguousarray(
                Wkv[:, c * D:(c + 1) * D].reshape(NKT, 128, D)
                .transpose(1, 0, 2).reshape(128, NKT * D)),
            "bk": np.ascontiguousarray(bkv[c * D:(c + 1) * D].reshape(128, 1)),
            "wv": np.concatenate([
                Wkv[:, kv_size + c * D:kv_size + (c + 1) * D]
                .reshape(NKT, 128, D).transpose(1, 0, 2).reshape(128, NKT * D),
                np.pad(bkv[kv_size + c * D:kv_size + (c + 1) * D]
                       .reshape(1, D), ((0, 0), (0, (NKT - 1) * D)))],
                axis=0),
            "wo": Wo,
            "kpT": kpT_h,
            "vpa": vpa_h,
            "dmask": dmask_h,
            "onem": np.ones((1, 128), np.float32),
            "csq1": csq1_h,
            "csq2": csq2_h,
            "css1": css1_h,
            "css2": css2_h,
            "swm": swm_h.astype(ml_dtypes.bfloat16),
            "swmf": swm_h,
            "idm": np.eye(D, dtype=np.float32),
            "sbias": sb_h,
        })

    res = bass_utils.run_bass_kernel_spmd(nc, in_maps,
                                          core_ids=list(range(N_CORES)))
    # core c's "out" rows: [0:256] = batch-0 tokens c*256.., [256:512] =
    # batch-1 tokens 2048 + c*256..
    half = TPC // B
    full = np.empty((T, HIDDEN), np.float32)
    for c in range(N_CORES):
        o = res.results[c]["out"]
        full[c * half:(c + 1) * half] = o[0:half]
        full[L + c * half:L + (c + 1) * half] = o[half:TPC]
    return full
